# revision 30
# baseline (speedup 1.0000x reference)
"""Causal self-attention (GPT-style block) on 8 Trainium2 NeuronCores.

Problem: x[4,2048,1024] -> qkv = x@W_attn+b ; 16-head causal attention
(head_dim 64) ; out = y@W_proj+b_proj.

Sharding: tensor-parallel over heads. Core c owns heads {2c, 2c+1}:
  - qkv projections from a host-pretransposed x^T (bf16); q^T/k^T stay
    feature-major for the S matmul; v is rotated to token-major layout
    on the PE via transpose-matmuls (no DMA transposes),
  - causal attention in S^T layout: scores [128 j-keys, 512 queries].
    Both heads of a j-tile share one 2-bank PSUM tile so a single
    ScalarE exp instruction covers them; diagonal masking via GpSimd
    affine_select; PV appends a ones-column to V so the softmax
    denominator falls out of the same PSUM tile. Normalization is
    deferred past the collective: raw y and the denominators travel
    together (130 bf16 rows per stripe).
  - per-batch striped AllToAll; emission is software-pipelined so batch
    b's collective+projection instructions sit behind batch b+1's
    attention in every engine queue (in-order queues never stall on an
    in-flight collective).

Numerics: bf16 operands, fp32 PSUM accumulation; softmax skips
max-subtraction (scores are O(1); exp stays finite).
"""

import numpy as np
import ml_dtypes
from contextlib import ExitStack

import concourse.bass as bass
import concourse.tile as tile
from concourse import bacc, mybir
from concourse.bass_utils import run_bass_kernel_spmd

F32 = mybir.dt.float32
BF16 = mybir.dt.bfloat16
AF = mybir.ActivationFunctionType

N_CORES = 8
B, T, C, H = 4, 2048, 1024, 16
HD = C // H            # 64 head dim
HPC = H // N_CORES     # 2 heads per core
FPC = HPC * HD         # 128 features per core
BT = B * T             # 8192 rows
TCHUNK = 512           # t chunk in qkv phase
NT_CHUNKS = BT // TCHUNK
CPB = T // TCHUNK      # 4 chunks per batch
QB = 512               # query block
NQB = T // QB          # 4 per batch
JTN = T // 128         # 16 j-tiles per batch
ROWS = BT // N_CORES   # 1024 rows per core after AllToAll
KC = C // 128          # 8 contraction tiles over C
STRIPE = 256           # rows per (core, batch) stripe
CCR = FPC + 2 * HPC    # A2A payload rows: y (128) + fp32 recip denoms (2x2 bf16)
SCALE = 1.0 / np.sqrt(HD)

LAST_RESULTS = None    # test.py reads exec_time_ns off this


def build_program(nc, debug=False):
    xT = nc.dram_tensor("xT", [C, BT], BF16, kind="ExternalInput").ap()
    wq = nc.dram_tensor("wq", [C, FPC], BF16, kind="ExternalInput").ap()
    wk = nc.dram_tensor("wk", [C, FPC], BF16, kind="ExternalInput").ap()
    wv = nc.dram_tensor("wv", [C, FPC], BF16, kind="ExternalInput").ap()
    bqkv = nc.dram_tensor("bqkv", [3, FPC], F32, kind="ExternalInput").ap()
    wp = nc.dram_tensor("wp", [C, C], BF16, kind="ExternalInput").ap()
    bp = nc.dram_tensor("bp", [C], F32, kind="ExternalInput").ap()
    ident = nc.dram_tensor("ident", [128, 128], BF16, kind="ExternalInput").ap()
    out = nc.dram_tensor("out", [ROWS, C], F32, kind="ExternalOutput").ap()
    cc_in = [
        nc.dram_tensor(f"cc_in{b}", [N_CORES, CCR, STRIPE], BF16, kind="Internal").ap()
        for b in range(B)
    ]
    cc_out = [
        nc.dram_tensor(f"cc_out{b}", [N_CORES, CCR, STRIPE], BF16, kind="Internal").ap()
        for b in range(B)
    ]

    dbg = None
    if debug:
        dbg = {
            "d_qT": nc.dram_tensor("d_qT", [128, BT], BF16, kind="ExternalOutput").ap(),
            "d_kT": nc.dram_tensor("d_kT", [128, BT], BF16, kind="ExternalOutput").ap(),
            "d_vsb": nc.dram_tensor(
                "d_vsb", [128, B * JTN, HPC, HD + 1], BF16, kind="ExternalOutput"
            ).ap(),
            "d_cc": nc.dram_tensor(
                "d_cc", [B, N_CORES, CCR, STRIPE], BF16, kind="ExternalOutput"
            ).ap(),
            "d_ccout": nc.dram_tensor(
                "d_ccout", [B, N_CORES, CCR, STRIPE], BF16, kind="ExternalOutput"
            ).ap(),
        }
    with tile.TileContext(nc) as tc:
        with ExitStack() as ctx:
            emit(ctx, tc, xT, wq, wk, wv, bqkv, wp, bp, ident, out, cc_in, cc_out, dbg)
    return nc


def emit(ctx, tc, xT, wq, wk, wv, bqkv, wp, bp, ident, out, cc_in, cc_out, dbg=None):
    nc = tc.nc
    res = ctx.enter_context(tc.tile_pool(name="resident", bufs=1))

    # ---------- resident SBUF ----------
    qT = res.tile([128, BT], BF16)
    kT = res.tile([128, BT], BF16)
    vsb = res.tile([128, B * JTN, HPC, HD + 2], BF16)  # v natural + ones col
    wq_sb = res.tile([128, KC, FPC], BF16)
    wk_sb = res.tile([128, KC, FPC], BF16)
    wv_sb = res.tile([128, KC, FPC], BF16)
    b_sb = res.tile([128, 3], F32)
    id_sb = res.tile([128, 128], BF16)
    wp_sb = res.tile([128, KC, C], BF16)
    bp_sb = res.tile([128, C], F32)

    nc.sync.dma_start(wq_sb[:], wq.rearrange("(a p) m -> p a m", p=128))
    nc.sync.dma_start(wk_sb[:], wk.rearrange("(a p) m -> p a m", p=128))
    nc.sync.dma_start(wv_sb[:], wv.rearrange("(a p) m -> p a m", p=128))
    nc.sync.dma_start(b_sb[:], bqkv.rearrange("b p -> p b"))
    nc.sync.dma_start(id_sb[:], ident)
    nc.vector.memset(vsb[:, :, :, HD : HD + 1], 1.0)

    # ---------- pools ----------
    # PSUM budget (8 banks): qkvps ring 2 (qkv chains + v transposes via
    # bitcast) + sp 4 (2-bank head-pair tiles, double-buffered) + ypool 2
    # (PV accumulators, reused as proj accumulators).
    xpool = ctx.enter_context(tc.tile_pool(name="xt", bufs=5))
    qkvps = ctx.enter_context(tc.tile_pool(name="qkvps", bufs=2, space="PSUM"))
    vstp = ctx.enter_context(tc.tile_pool(name="vst", bufs=5))
    spool = ctx.enter_context(tc.tile_pool(name="sps", bufs=2, space="PSUM"))
    ypool = ctx.enter_context(tc.tile_pool(name="yps", bufs=1, space="PSUM"))
    ptpool = ctx.enter_context(tc.tile_pool(name="pt", bufs=4))
    stpool = ctx.enter_context(tc.tile_pool(name="stg", bufs=3))
    ospool = ctx.enter_context(tc.tile_pool(name="osb", bufs=2))
    yfpool = ctx.enter_context(tc.tile_pool(name="yf", bufs=2))

    xT_t = xT.rearrange("(a p) t -> p a t", p=128)
    pend_trans = []  # deferred v transposes: (vst tile, chunk idx)

    def qkv_chunk(tci):
        """qkv projections for one 512-token chunk; v transposes deferred."""
        t0 = tci * TCHUNK
        xt = xpool.tile([128, KC, TCHUNK], BF16, tag="xt")
        nspl = 8 if tci == 0 else 4
        w = KC // nspl
        for spl in range(nspl):
            eng = nc.sync if spl % 2 == 0 else nc.scalar
            eng.dma_start(
                xt[:, w * spl : w * (spl + 1), :],
                xT_t[:, w * spl : w * (spl + 1), t0 : t0 + TCHUNK],
            )
        for w_sb, bi, dst in ((wq_sb, 0, qT), (wk_sb, 1, kT), (wv_sb, 2, None)):
            ps = qkvps.tile([128, TCHUNK], F32, tag="qkvps")
            for a in range(KC):
                nc.tensor.matmul(
                    ps[:], lhsT=w_sb[:, a, :], rhs=xt[:, a, :],
                    start=(a == 0), stop=(a == KC - 1),
                )
            if dst is not None:
                nc.vector.tensor_scalar_add(
                    dst[:, t0 : t0 + TCHUNK], ps[:], b_sb[:, bi : bi + 1]
                )
            else:
                vst = vstp.tile([128, TCHUNK], BF16, tag="vst")
                nc.vector.tensor_scalar_add(vst[:], ps[:], b_sb[:, bi : bi + 1])
                pend_trans.append((vst, tci))

    def flush_trans():
        """PE-transpose pending v chunks into vsb (token-major)."""
        while pend_trans:
            vst, tci = pend_trans.pop(0)
            tpf = qkvps.tile([128, TCHUNK], F32, tag="qkvps")
            tp = tpf.bitcast(BF16)  # [128, 1024] bf16 view; use first 512
            for g4 in range(4):
                g = 4 * tci + g4
                nc.tensor.transpose(
                    tp[:, g4 * 128 : (g4 + 1) * 128],
                    vst[:, g4 * 128 : (g4 + 1) * 128], id_sb[:]
                )
                nc.vector.tensor_copy(
                    vsb[:, g, 0:HPC, 0:HD], tp[:, g4 * 128 : (g4 + 1) * 128]
                )

    def attention_batch(b):
        for qb in range(NQB):
            q0g = b * T + qb * QB
            njt = 4 * (qb + 1)
            yps = [
                ypool.tile([128, QB], F32, tag=f"yps{h}", name=f"yp{b}_{qb}_{h}")
                for h in range(HPC)
            ]
            for j in range(njt):
                j0g = b * T + j * 128
                i0 = max(0, j * 128 - qb * QB)
                diag = j * 128 + 127 > qb * QB
                sp = spool.tile([128, HPC, QB], F32, tag="sp")
                pt = ptpool.tile([128, HPC, QB], BF16, tag="pt")
                for h in range(HPC):
                    hs = slice(h * HD, (h + 1) * HD)
                    nc.tensor.matmul(
                        sp[:, h, i0:QB], lhsT=kT[hs, j0g : j0g + 128],
                        rhs=qT[hs, q0g + i0 : q0g + QB], start=True, stop=True,
                    )
                # one exp covers both heads (identical i0 geometry)
                nc.scalar.activation(
                    pt[:, :, i0:QB], sp[:, :, i0:QB], AF.Exp, scale=float(SCALE)
                )
                for h in range(HPC):
                    if diag:
                        nc.gpsimd.affine_select(
                            pt[:, h, i0 : i0 + 128], pt[:, h, i0 : i0 + 128],
                            pattern=[[1, 128]], base=0, channel_multiplier=-1,
                            compare_op=mybir.AluOpType.is_ge, fill=0.0,
                        )
                    nc.tensor.matmul(
                        yps[h][0 : HD + 1, i0:QB],
                        lhsT=vsb[:, b * JTN + j, h, 0 : HD + 1],
                        rhs=pt[:, h, i0:QB],
                        start=(j == 0), stop=(j == njt - 1),
                        skip_group_check=True,
                    )
            # evict raw y, reciprocal the denominators, stage for the collective
            yst = stpool.tile([FPC, QB], BF16, tag="yst")
            ln = stpool.tile([1, HPC, QB], F32, tag="ln", bufs=1)
            for h in range(HPC):
                nc.vector.tensor_copy(yst[h * HD : (h + 1) * HD, :], yps[h][0:HD, :])
                nc.vector.tensor_copy(ln[0:1, h, :], yps[h][HD : HD + 1, :])
            rnl = stpool.tile([1, HPC, QB], F32, tag="rnl", bufs=1)
            scr = stpool.tile([1, HPC, QB], F32, tag="scr", bufs=1)
            nc.vector.reciprocal_approx_accurate(rnl[:], ln[:], scr[:])
            rnb = rnl.bitcast(BF16)  # [1, HPC, 2*QB]
            for s in range(2):
                r = 2 * qb + s
                nc.gpsimd.dma_start(
                    cc_in[b][r, 0:FPC, :], yst[:, s * STRIPE : (s + 1) * STRIPE]
                )
                # fp32 recips ride as 2 bf16 rows per head
                nc.sync.dma_start(
                    cc_in[b][r, FPC:CCR, :], rnb[0:1, :, s * QB : (s + 1) * QB]
                )
        nc.gpsimd.collective_compute(
            "AllToAll", mybir.AluOpType.bypass,
            ins=[cc_in[b][:]], outs=[cc_out[b][:]],
            replica_groups=[list(range(N_CORES))],
        )

    def project_batch(b):
        yfull = yfpool.tile([128, KC, STRIPE], BF16, tag="yf")
        nc.sync.dma_start(yfull[:], cc_out[b][:, 0:FPC, :].rearrange("r p t -> p r t"))
        # rebuild the per-(feature, t) scale tile by broadcast-DMAing the
        # fp32 reciprocals straight out of cc_out (partition-stride-0 APs)
        sc = yfpool.tile([128, KC, STRIPE], F32, tag="sc", bufs=1)
        for r in range(N_CORES):
            for h in range(HPC):
                row = cc_out[b][r, FPC + 2 * h : FPC + 2 * h + 2, :]
                rowf = row.rearrange("h t -> (h t)").bitcast(F32)  # [STRIPE] f32
                src = bass.AP(
                    tensor=rowf.tensor, offset=rowf.offset,
                    ap=[[0, HD]] + [list(p) for p in rowf.ap],
                )
                nc.gpsimd.dma_start(sc[h * HD : (h + 1) * HD, r, :], src)
        nc.vector.tensor_mul(yfull[:], yfull[:], sc[:])
        for tt in range(STRIPE // 128):
            ps0 = ypool.tile([128, 512], F32, tag="yps0", name=f"pj{b}_{tt}_0")
            ps1 = ypool.tile([128, 512], F32, tag="yps1", name=f"pj{b}_{tt}_1")
            for a in range(KC):
                lhsT = yfull[:, a, tt * 128 : (tt + 1) * 128]
                nc.tensor.matmul(ps0[:], lhsT=lhsT, rhs=wp_sb[:, a, 0:512],
                                 start=(a == 0), stop=(a == KC - 1))
                nc.tensor.matmul(ps1[:], lhsT=lhsT, rhs=wp_sb[:, a, 512:C],
                                 start=(a == 0), stop=(a == KC - 1))
            osb = ospool.tile([128, C], F32, tag="osb")
            nc.vector.tensor_add(osb[:, 0:512], ps0[:], bp_sb[:, 0:512])
            nc.vector.tensor_add(osb[:, 512:C], ps1[:], bp_sb[:, 512:C])
            r0 = b * STRIPE + tt * 128
            nc.sync.dma_start(out[r0 : r0 + 128, :], osb[:])

    # ---------- software-pipelined emission ----------
    # qkv(b) -> [attention(b-1) interleaves via deps] ; proj(b) sits behind
    # attention(b+1) so no engine queue waits on an in-flight collective.
    for b in range(B):
        for tci in range(CPB * b, CPB * (b + 1)):
            qkv_chunk(tci)
        flush_trans()
        if b == 0:
            # deferred weight loads: keep the early DMA queues clear for x
            nc.sync.dma_start(wp_sb[:], wp.rearrange("(a p) e -> p a e", p=128))
            bp_bcast = bass.AP(
                tensor=bp.tensor, offset=bp.offset, ap=[[0, 128], [1, C]]
            )
            nc.sync.dma_start(bp_sb[:], bp_bcast)
        if b >= 2:
            project_batch(b - 2)
        if b >= 1:
            attention_batch(b - 1)
    project_batch(B - 2)
    attention_batch(B - 1)
    project_batch(B - 1)

    if dbg is not None:
        nc.sync.dma_start(dbg["d_qT"][:], qT[:])
        nc.sync.dma_start(dbg["d_kT"][:], kT[:])
        nc.sync.dma_start(dbg["d_vsb"][:], vsb[:, :, :, 0 : HD + 1])
        for b in range(B):
            nc.sync.dma_start(dbg["d_cc"][b], cc_in[b][:])
            nc.sync.dma_start(dbg["d_ccout"][b], cc_out[b][:])


_COMPILED_NC = None


def _get_nc():
    global _COMPILED_NC
    if _COMPILED_NC is None:
        nc = bacc.Bacc("TRN2", target_bir_lowering=False, debug=False,
                       num_devices=N_CORES)
        build_program(nc)
        nc.compile()
        _COMPILED_NC = nc
    return _COMPILED_NC


def kernel(x, W_attn, b_attn, W_proj, b_proj):
    global LAST_RESULTS
    nc = _get_nc()

    bf = ml_dtypes.bfloat16
    xT_np = np.ascontiguousarray(
        np.asarray(x, np.float32).reshape(BT, C).T
    ).astype(bf)
    W_attn = np.asarray(W_attn, np.float32)
    b_attn = np.asarray(b_attn, np.float32)
    wp_np = np.asarray(W_proj, np.float32).astype(bf)
    bp_np = np.asarray(b_proj, np.float32)
    id_np = np.eye(128, dtype=np.float32).astype(bf)

    in_maps = []
    for c in range(N_CORES):
        s = slice(c * FPC, (c + 1) * FPC)
        in_maps.append({
            "xT": xT_np,
            "wq": np.ascontiguousarray(W_attn[:, s]).astype(bf),
            "wk": np.ascontiguousarray(W_attn[:, C:2 * C][:, s]).astype(bf),
            "wv": np.ascontiguousarray(W_attn[:, 2 * C:][:, s]).astype(bf),
            "bqkv": np.ascontiguousarray(
                np.stack([b_attn[s], b_attn[C:2 * C][s], b_attn[2 * C:][s]])
            ).astype(np.float32),
            "wp": wp_np,
            "bp": bp_np,
            "ident": id_np,
        })

    res = run_bass_kernel_spmd(nc, in_maps, core_ids=list(range(N_CORES)))
    LAST_RESULTS = res
    # core r, out row b*256+i  ->  full row b*2048 + r*256 + i
    outs = np.stack([res.results[c]["out"] for c in range(N_CORES)])
    full = outs.reshape(N_CORES, B, STRIPE, C).transpose(1, 0, 2, 3)
    return np.ascontiguousarray(full.reshape(B, T, C))


# revision 31
# speedup vs baseline: 1.1985x; 1.1985x over previous
"""Causal self-attention (GPT-style block) on 8 Trainium2 NeuronCores.

Problem: x[4,2048,1024] -> qkv = x@W_attn+b ; 16-head causal attention
(head_dim 64) ; out = y@W_proj+b_proj.

Sharding: tensor-parallel over heads. Core c owns heads {2c, 2c+1}:
  - qkv projections from a host-pretransposed x^T (bf16); q^T/k^T stay
    feature-major for the S matmul; v is rotated to token-major layout
    on the PE via transpose-matmuls (no DMA transposes),
  - causal attention in S^T layout: scores [128 j-keys, 512 queries].
    Both heads of a j-tile share one 2-bank PSUM tile so a single
    ScalarE exp instruction covers them; diagonal masking via GpSimd
    affine_select; PV appends a ones-column to V so the softmax
    denominator falls out of the same PSUM tile. Normalization is
    deferred past the collective: raw y and the denominators travel
    together (130 bf16 rows per stripe).
  - per-batch striped AllToAll; emission is software-pipelined so batch
    b's collective+projection instructions sit behind batch b+1's
    attention in every engine queue (in-order queues never stall on an
    in-flight collective).

Numerics: bf16 operands, fp32 PSUM accumulation; softmax skips
max-subtraction (scores are O(1); exp stays finite).
"""

import numpy as np
import ml_dtypes
from contextlib import ExitStack

import concourse.bass as bass
import concourse.tile as tile
from concourse import bacc, mybir
from concourse.bass_utils import run_bass_kernel_spmd

F32 = mybir.dt.float32
BF16 = mybir.dt.bfloat16
AF = mybir.ActivationFunctionType

N_CORES = 8
B, T, C, H = 4, 2048, 1024, 16
HD = C // H            # 64 head dim
HPC = H // N_CORES     # 2 heads per core
FPC = HPC * HD         # 128 features per core
BT = B * T             # 8192 rows
TCHUNK = 512           # t chunk in qkv phase
NT_CHUNKS = BT // TCHUNK
CPB = T // TCHUNK      # 4 chunks per batch
QB = 512               # query block
NQB = T // QB          # 4 per batch
JTN = T // 128         # 16 j-tiles per batch
ROWS = BT // N_CORES   # 1024 rows per core after AllToAll
KC = C // 128          # 8 contraction tiles over C
STRIPE = 256           # rows per (core, batch) stripe
CCR = FPC + 2 * HPC    # A2A payload rows: y (128) + fp32 recip denoms (2x2 bf16)
SCALE = 1.0 / np.sqrt(HD)

LAST_RESULTS = None    # test.py reads exec_time_ns off this


def build_program(nc, debug=False):
    xT = nc.dram_tensor("xT", [C, BT], BF16, kind="ExternalInput").ap()
    wq = nc.dram_tensor("wq", [C, FPC], BF16, kind="ExternalInput").ap()
    wk = nc.dram_tensor("wk", [C, FPC], BF16, kind="ExternalInput").ap()
    wv = nc.dram_tensor("wv", [C, FPC], BF16, kind="ExternalInput").ap()
    bqkv = nc.dram_tensor("bqkv", [3, FPC], F32, kind="ExternalInput").ap()
    wp = nc.dram_tensor("wp", [C, C], BF16, kind="ExternalInput").ap()
    bp = nc.dram_tensor("bp", [C], F32, kind="ExternalInput").ap()
    ident = nc.dram_tensor("ident", [128, 128], BF16, kind="ExternalInput").ap()
    out = nc.dram_tensor("out", [ROWS, C], F32, kind="ExternalOutput").ap()
    cc_in = [
        nc.dram_tensor(f"cc_in{b}", [N_CORES, CCR, STRIPE], BF16, kind="Internal").ap()
        for b in range(B)
    ]
    cc_out = [
        nc.dram_tensor(f"cc_out{b}", [N_CORES, CCR, STRIPE], BF16, kind="Internal").ap()
        for b in range(B)
    ]

    dbg = None
    if debug:
        dbg = {
            "d_qT": nc.dram_tensor("d_qT", [128, BT], BF16, kind="ExternalOutput").ap(),
            "d_kT": nc.dram_tensor("d_kT", [128, BT], BF16, kind="ExternalOutput").ap(),
            "d_vsb": nc.dram_tensor(
                "d_vsb", [128, B * JTN, HPC, HD + 1], BF16, kind="ExternalOutput"
            ).ap(),
            "d_cc": nc.dram_tensor(
                "d_cc", [B, N_CORES, CCR, STRIPE], BF16, kind="ExternalOutput"
            ).ap(),
            "d_ccout": nc.dram_tensor(
                "d_ccout", [B, N_CORES, CCR, STRIPE], BF16, kind="ExternalOutput"
            ).ap(),
        }
    with tile.TileContext(nc) as tc:
        with ExitStack() as ctx:
            emit(ctx, tc, xT, wq, wk, wv, bqkv, wp, bp, ident, out, cc_in, cc_out, dbg)
    return nc


def emit(ctx, tc, xT, wq, wk, wv, bqkv, wp, bp, ident, out, cc_in, cc_out, dbg=None):
    nc = tc.nc
    res = ctx.enter_context(tc.tile_pool(name="resident", bufs=1))

    # ---------- resident SBUF ----------
    qT = res.tile([128, BT], BF16)
    kT = res.tile([128, BT], BF16)
    vsb = res.tile([128, B * JTN, HPC, HD + 2], BF16)  # v natural + ones col
    wq_sb = res.tile([128, KC, FPC], BF16)
    wk_sb = res.tile([128, KC, FPC], BF16)
    wv_sb = res.tile([128, KC, FPC], BF16)
    b_sb = res.tile([128, 3], F32)
    id_sb = res.tile([128, 128], BF16)
    wp_sb = res.tile([128, KC, C], BF16)
    bp_sb = res.tile([128, C], F32)

    nc.sync.dma_start(wq_sb[:], wq.rearrange("(a p) m -> p a m", p=128))
    nc.sync.dma_start(wk_sb[:], wk.rearrange("(a p) m -> p a m", p=128))
    nc.sync.dma_start(wv_sb[:], wv.rearrange("(a p) m -> p a m", p=128))
    nc.sync.dma_start(b_sb[:], bqkv.rearrange("b p -> p b"))
    nc.sync.dma_start(id_sb[:], ident)
    nc.vector.memset(vsb[:, :, :, HD : HD + 1], 1.0)

    # ---------- pools ----------
    # PSUM budget (8 banks): qkvps ring 2 (qkv chains + v transposes via
    # bitcast) + sp 4 (2-bank head-pair tiles, double-buffered) + ypool 2
    # (PV accumulators, reused as proj accumulators).
    xpool = ctx.enter_context(tc.tile_pool(name="xt", bufs=5))
    qkvps = ctx.enter_context(tc.tile_pool(name="qkvps", bufs=2, space="PSUM"))
    vstp = ctx.enter_context(tc.tile_pool(name="vst", bufs=5))
    spool = ctx.enter_context(tc.tile_pool(name="sps", bufs=2, space="PSUM"))
    ypool = ctx.enter_context(tc.tile_pool(name="yps", bufs=1, space="PSUM"))
    ptpool = ctx.enter_context(tc.tile_pool(name="pt", bufs=4))
    stpool = ctx.enter_context(tc.tile_pool(name="stg", bufs=3))
    ospool = ctx.enter_context(tc.tile_pool(name="osb", bufs=2))
    yfpool = ctx.enter_context(tc.tile_pool(name="yf", bufs=2))

    xT_t = xT.rearrange("(a p) t -> p a t", p=128)
    pend_trans = []  # deferred v transposes: (vst tile, chunk idx)

    def qkv_chunk(tci):
        """qkv projections for one 512-token chunk; v transposes deferred."""
        t0 = tci * TCHUNK
        xt = xpool.tile([128, KC, TCHUNK], BF16, tag="xt")
        nspl = 8 if tci == 0 else 4
        w = KC // nspl
        for spl in range(nspl):
            eng = nc.sync if spl % 2 == 0 else nc.scalar
            eng.dma_start(
                xt[:, w * spl : w * (spl + 1), :],
                xT_t[:, w * spl : w * (spl + 1), t0 : t0 + TCHUNK],
            )
        for w_sb, bi, dst in ((wq_sb, 0, qT), (wk_sb, 1, kT), (wv_sb, 2, None)):
            ps = qkvps.tile([128, TCHUNK], F32, tag="qkvps")
            for a in range(KC):
                nc.tensor.matmul(
                    ps[:], lhsT=w_sb[:, a, :], rhs=xt[:, a, :],
                    start=(a == 0), stop=(a == KC - 1),
                )
            if dst is not None:
                nc.vector.tensor_scalar_add(
                    dst[:, t0 : t0 + TCHUNK], ps[:], b_sb[:, bi : bi + 1]
                )
            else:
                vst = vstp.tile([128, TCHUNK], BF16, tag="vst")
                nc.vector.tensor_scalar_add(vst[:], ps[:], b_sb[:, bi : bi + 1])
                pend_trans.append((vst, tci))

    def flush_trans():
        """PE-transpose pending v chunks into vsb (token-major)."""
        while pend_trans:
            vst, tci = pend_trans.pop(0)
            tpf = qkvps.tile([128, TCHUNK], F32, tag="qkvps")
            tp = tpf.bitcast(BF16)  # [128, 1024] bf16 view; use first 512
            for g4 in range(4):
                g = 4 * tci + g4
                nc.tensor.transpose(
                    tp[:, g4 * 128 : (g4 + 1) * 128],
                    vst[:, g4 * 128 : (g4 + 1) * 128], id_sb[:]
                )
                nc.vector.tensor_copy(
                    vsb[:, g, 0:HPC, 0:HD], tp[:, g4 * 128 : (g4 + 1) * 128]
                )

    def attention_batch(b):
        for qb in range(NQB):
            q0g = b * T + qb * QB
            njt = 4 * (qb + 1)
            yps = [
                ypool.tile([128, QB], F32, tag=f"yps{h}", name=f"yp{b}_{qb}_{h}")
                for h in range(HPC)
            ]
            for j in range(njt):
                j0g = b * T + j * 128
                i0 = max(0, j * 128 - qb * QB)
                diag = j * 128 + 127 > qb * QB
                sp = spool.tile([128, HPC, QB], F32, tag="sp")
                pt = ptpool.tile([128, HPC, QB], BF16, tag="pt")
                for h in range(HPC):
                    hs = slice(h * HD, (h + 1) * HD)
                    nc.tensor.matmul(
                        sp[:, h, i0:QB], lhsT=kT[hs, j0g : j0g + 128],
                        rhs=qT[hs, q0g + i0 : q0g + QB], start=True, stop=True,
                    )
                # one exp covers both heads (identical i0 geometry)
                nc.scalar.activation(
                    pt[:, :, i0:QB], sp[:, :, i0:QB], AF.Exp, scale=float(SCALE)
                )
                for h in range(HPC):
                    if diag:
                        nc.gpsimd.affine_select(
                            pt[:, h, i0 : i0 + 128], pt[:, h, i0 : i0 + 128],
                            pattern=[[1, 128]], base=0, channel_multiplier=-1,
                            compare_op=mybir.AluOpType.is_ge, fill=0.0,
                        )
                    nc.tensor.matmul(
                        yps[h][0 : HD + 1, i0:QB],
                        lhsT=vsb[:, b * JTN + j, h, 0 : HD + 1],
                        rhs=pt[:, h, i0:QB],
                        start=(j == 0), stop=(j == njt - 1),
                        skip_group_check=True,
                    )
            # evict raw y, reciprocal the denominators, stage for the collective
            yst = stpool.tile([FPC, QB], BF16, tag="yst")
            ln = stpool.tile([1, HPC, QB], F32, tag="ln", bufs=1)
            for h in range(HPC):
                nc.vector.tensor_copy(yst[h * HD : (h + 1) * HD, :], yps[h][0:HD, :])
                nc.vector.tensor_copy(ln[0:1, h, :], yps[h][HD : HD + 1, :])
            rnl = stpool.tile([1, HPC, QB], F32, tag="rnl", bufs=1)
            scr = stpool.tile([1, HPC, QB], F32, tag="scr", bufs=1)
            nc.vector.reciprocal_approx_accurate(rnl[:], ln[:], scr[:])
            rnb = rnl.bitcast(BF16)  # [1, HPC, 2*QB]
            for s in range(2):
                r = 2 * qb + s
                nc.sync.dma_start(
                    cc_in[b][r, 0:FPC, :], yst[:, s * STRIPE : (s + 1) * STRIPE]
                )
                # fp32 recips ride as 2 bf16 rows per head
                nc.sync.dma_start(
                    cc_in[b][r, FPC:CCR, :], rnb[0:1, :, s * QB : (s + 1) * QB]
                )
        nc.gpsimd.collective_compute(
            "AllToAll", mybir.AluOpType.bypass,
            ins=[cc_in[b][:]], outs=[cc_out[b][:]],
            replica_groups=[list(range(N_CORES))],
        )

    def project_batch(b):
        yfull = yfpool.tile([128, KC, STRIPE], BF16, tag="yf")
        nc.sync.dma_start(yfull[:], cc_out[b][:, 0:FPC, :].rearrange("r p t -> p r t"))
        # rebuild the per-(feature, t) scale tile by broadcast-DMAing the
        # fp32 reciprocals straight out of cc_out (partition-stride-0 APs)
        sc = yfpool.tile([128, KC, STRIPE], F32, tag="sc", bufs=1)
        for r in range(N_CORES):
            for h in range(HPC):
                row = cc_out[b][r, FPC + 2 * h : FPC + 2 * h + 2, :]
                rowf = row.rearrange("h t -> (h t)").bitcast(F32)  # [STRIPE] f32
                src = bass.AP(
                    tensor=rowf.tensor, offset=rowf.offset,
                    ap=[[0, HD]] + [list(p) for p in rowf.ap],
                )
                nc.sync.dma_start(sc[h * HD : (h + 1) * HD, r, :], src)
        nc.vector.tensor_mul(yfull[:], yfull[:], sc[:])
        for tt in range(STRIPE // 128):
            ps0 = ypool.tile([128, 512], F32, tag="yps0", name=f"pj{b}_{tt}_0")
            ps1 = ypool.tile([128, 512], F32, tag="yps1", name=f"pj{b}_{tt}_1")
            for a in range(KC):
                lhsT = yfull[:, a, tt * 128 : (tt + 1) * 128]
                nc.tensor.matmul(ps0[:], lhsT=lhsT, rhs=wp_sb[:, a, 0:512],
                                 start=(a == 0), stop=(a == KC - 1))
                nc.tensor.matmul(ps1[:], lhsT=lhsT, rhs=wp_sb[:, a, 512:C],
                                 start=(a == 0), stop=(a == KC - 1))
            osb = ospool.tile([128, C], F32, tag="osb")
            nc.vector.tensor_add(osb[:, 0:512], ps0[:], bp_sb[:, 0:512])
            nc.vector.tensor_add(osb[:, 512:C], ps1[:], bp_sb[:, 512:C])
            r0 = b * STRIPE + tt * 128
            nc.sync.dma_start(out[r0 : r0 + 128, :], osb[:])

    # ---------- software-pipelined emission ----------
    # qkv(b) -> [attention(b-1) interleaves via deps] ; proj(b) sits behind
    # attention(b+1) so no engine queue waits on an in-flight collective.
    for b in range(B):
        for tci in range(CPB * b, CPB * (b + 1)):
            qkv_chunk(tci)
        flush_trans()
        if b == 0:
            # deferred weight loads: keep the early DMA queues clear for x
            nc.sync.dma_start(wp_sb[:], wp.rearrange("(a p) e -> p a e", p=128))
            bp_bcast = bass.AP(
                tensor=bp.tensor, offset=bp.offset, ap=[[0, 128], [1, C]]
            )
            nc.sync.dma_start(bp_sb[:], bp_bcast)
        if b >= 1:
            attention_batch(b - 1)
        if b >= 2:
            project_batch(b - 2)
    attention_batch(B - 1)
    project_batch(B - 2)
    project_batch(B - 1)

    if dbg is not None:
        nc.sync.dma_start(dbg["d_qT"][:], qT[:])
        nc.sync.dma_start(dbg["d_kT"][:], kT[:])
        nc.sync.dma_start(dbg["d_vsb"][:], vsb[:, :, :, 0 : HD + 1])
        for b in range(B):
            nc.sync.dma_start(dbg["d_cc"][b], cc_in[b][:])
            nc.sync.dma_start(dbg["d_ccout"][b], cc_out[b][:])


_COMPILED_NC = None


def _get_nc():
    global _COMPILED_NC
    if _COMPILED_NC is None:
        nc = bacc.Bacc("TRN2", target_bir_lowering=False, debug=False,
                       num_devices=N_CORES)
        build_program(nc)
        nc.compile()
        _COMPILED_NC = nc
    return _COMPILED_NC


def kernel(x, W_attn, b_attn, W_proj, b_proj):
    global LAST_RESULTS
    nc = _get_nc()

    bf = ml_dtypes.bfloat16
    xT_np = np.ascontiguousarray(
        np.asarray(x, np.float32).reshape(BT, C).T
    ).astype(bf)
    W_attn = np.asarray(W_attn, np.float32)
    b_attn = np.asarray(b_attn, np.float32)
    wp_np = np.asarray(W_proj, np.float32).astype(bf)
    bp_np = np.asarray(b_proj, np.float32)
    id_np = np.eye(128, dtype=np.float32).astype(bf)

    in_maps = []
    for c in range(N_CORES):
        s = slice(c * FPC, (c + 1) * FPC)
        in_maps.append({
            "xT": xT_np,
            "wq": np.ascontiguousarray(W_attn[:, s]).astype(bf),
            "wk": np.ascontiguousarray(W_attn[:, C:2 * C][:, s]).astype(bf),
            "wv": np.ascontiguousarray(W_attn[:, 2 * C:][:, s]).astype(bf),
            "bqkv": np.ascontiguousarray(
                np.stack([b_attn[s], b_attn[C:2 * C][s], b_attn[2 * C:][s]])
            ).astype(np.float32),
            "wp": wp_np,
            "bp": bp_np,
            "ident": id_np,
        })

    res = run_bass_kernel_spmd(nc, in_maps, core_ids=list(range(N_CORES)))
    LAST_RESULTS = res
    # core r, out row b*256+i  ->  full row b*2048 + r*256 + i
    outs = np.stack([res.results[c]["out"] for c in range(N_CORES)])
    full = outs.reshape(N_CORES, B, STRIPE, C).transpose(1, 0, 2, 3)
    return np.ascontiguousarray(full.reshape(B, T, C))


# revision 32
# speedup vs baseline: 1.2034x; 1.0041x over previous
"""Causal self-attention (GPT-style block) on 8 Trainium2 NeuronCores.

Problem: x[4,2048,1024] -> qkv = x@W_attn+b ; 16-head causal attention
(head_dim 64) ; out = y@W_proj+b_proj.

Sharding: tensor-parallel over heads. Core c owns heads {2c, 2c+1}:
  - qkv projections from a host-pretransposed x^T (bf16); q^T/k^T stay
    feature-major for the S matmul; v is rotated to token-major layout
    on the PE via transpose-matmuls (no DMA transposes),
  - causal attention in S^T layout: scores [128 j-keys, 512 queries].
    Both heads of a j-tile share one 2-bank PSUM tile so a single
    ScalarE exp instruction covers them; diagonal masking via GpSimd
    affine_select; PV appends a ones-column to V so the softmax
    denominator falls out of the same PSUM tile. Normalization is
    deferred past the collective: raw y and the denominators travel
    together (130 bf16 rows per stripe).
  - per-batch striped AllToAll; emission is software-pipelined so batch
    b's collective+projection instructions sit behind batch b+1's
    attention in every engine queue (in-order queues never stall on an
    in-flight collective).

Numerics: bf16 operands, fp32 PSUM accumulation; softmax skips
max-subtraction (scores are O(1); exp stays finite).
"""

import numpy as np
import ml_dtypes
from contextlib import ExitStack

import concourse.bass as bass
import concourse.tile as tile
from concourse import bacc, mybir
from concourse.bass_utils import run_bass_kernel_spmd

F32 = mybir.dt.float32
BF16 = mybir.dt.bfloat16
AF = mybir.ActivationFunctionType

N_CORES = 8
B, T, C, H = 4, 2048, 1024, 16
HD = C // H            # 64 head dim
HPC = H // N_CORES     # 2 heads per core
FPC = HPC * HD         # 128 features per core
BT = B * T             # 8192 rows
TCHUNK = 512           # t chunk in qkv phase
NT_CHUNKS = BT // TCHUNK
CPB = T // TCHUNK      # 4 chunks per batch
QB = 512               # query block
NQB = T // QB          # 4 per batch
JTN = T // 128         # 16 j-tiles per batch
ROWS = BT // N_CORES   # 1024 rows per core after AllToAll
KC = C // 128          # 8 contraction tiles over C
STRIPE = 256           # rows per (core, batch) stripe
CCR = FPC + 2 * HPC    # A2A payload rows: y (128) + fp32 recip denoms (2x2 bf16)
SCALE = 1.0 / np.sqrt(HD)

LAST_RESULTS = None    # test.py reads exec_time_ns off this


def build_program(nc, debug=False):
    xT = nc.dram_tensor("xT", [C, BT], BF16, kind="ExternalInput").ap()
    wq = nc.dram_tensor("wq", [C, FPC], BF16, kind="ExternalInput").ap()
    wk = nc.dram_tensor("wk", [C, FPC], BF16, kind="ExternalInput").ap()
    wv = nc.dram_tensor("wv", [C, FPC], BF16, kind="ExternalInput").ap()
    bqkv = nc.dram_tensor("bqkv", [3, FPC], F32, kind="ExternalInput").ap()
    wp = nc.dram_tensor("wp", [C, C], BF16, kind="ExternalInput").ap()
    bp = nc.dram_tensor("bp", [C], F32, kind="ExternalInput").ap()
    ident = nc.dram_tensor("ident", [128, 128], BF16, kind="ExternalInput").ap()
    out = nc.dram_tensor("out", [ROWS, C], F32, kind="ExternalOutput").ap()
    cc_in = [
        nc.dram_tensor(f"cc_in{b}", [N_CORES, CCR, STRIPE], BF16, kind="Internal").ap()
        for b in range(B)
    ]
    cc_out = [
        nc.dram_tensor(f"cc_out{b}", [N_CORES, CCR, STRIPE], BF16, kind="Internal").ap()
        for b in range(B)
    ]

    dbg = None
    if debug:
        dbg = {
            "d_qT": nc.dram_tensor("d_qT", [128, BT], BF16, kind="ExternalOutput").ap(),
            "d_kT": nc.dram_tensor("d_kT", [128, BT], BF16, kind="ExternalOutput").ap(),
            "d_vsb": nc.dram_tensor(
                "d_vsb", [128, B * JTN, HPC, HD + 1], BF16, kind="ExternalOutput"
            ).ap(),
            "d_cc": nc.dram_tensor(
                "d_cc", [B, N_CORES, CCR, STRIPE], BF16, kind="ExternalOutput"
            ).ap(),
            "d_ccout": nc.dram_tensor(
                "d_ccout", [B, N_CORES, CCR, STRIPE], BF16, kind="ExternalOutput"
            ).ap(),
        }
    with tile.TileContext(nc) as tc:
        with ExitStack() as ctx:
            emit(ctx, tc, xT, wq, wk, wv, bqkv, wp, bp, ident, out, cc_in, cc_out, dbg)
    return nc


def emit(ctx, tc, xT, wq, wk, wv, bqkv, wp, bp, ident, out, cc_in, cc_out, dbg=None):
    nc = tc.nc
    res = ctx.enter_context(tc.tile_pool(name="resident", bufs=1))

    # ---------- resident SBUF ----------
    qT = res.tile([128, BT], BF16)
    kT = res.tile([128, BT], BF16)
    vsb = res.tile([128, B * JTN, HPC, HD + 2], BF16)  # v natural + ones col
    wq_sb = res.tile([128, KC, FPC], BF16)
    wk_sb = res.tile([128, KC, FPC], BF16)
    wv_sb = res.tile([128, KC, FPC], BF16)
    b_sb = res.tile([128, 3], F32)
    id_sb = res.tile([128, 128], BF16)
    wp_sb = res.tile([128, KC, C], BF16)
    bp_sb = res.tile([128, C], F32)

    nc.sync.dma_start(wq_sb[:], wq.rearrange("(a p) m -> p a m", p=128))
    nc.sync.dma_start(wk_sb[:], wk.rearrange("(a p) m -> p a m", p=128))
    nc.sync.dma_start(wv_sb[:], wv.rearrange("(a p) m -> p a m", p=128))
    nc.sync.dma_start(b_sb[:], bqkv.rearrange("b p -> p b"))
    nc.sync.dma_start(id_sb[:], ident)
    nc.vector.memset(vsb[:, :, :, HD : HD + 1], 1.0)

    # ---------- pools ----------
    # PSUM budget (8 banks): qkvps ring 2 (qkv chains + v transposes via
    # bitcast) + sp 4 (2-bank head-pair tiles, double-buffered) + ypool 2
    # (PV accumulators, reused as proj accumulators).
    xpool = ctx.enter_context(tc.tile_pool(name="xt", bufs=5))
    qkvps = ctx.enter_context(tc.tile_pool(name="qkvps", bufs=2, space="PSUM"))
    vstp = ctx.enter_context(tc.tile_pool(name="vst", bufs=5))
    spool = ctx.enter_context(tc.tile_pool(name="sps", bufs=2, space="PSUM"))
    ypool = ctx.enter_context(tc.tile_pool(name="yps", bufs=1, space="PSUM"))
    ptpool = ctx.enter_context(tc.tile_pool(name="pt", bufs=4))
    stpool = ctx.enter_context(tc.tile_pool(name="stg", bufs=3))
    ospool = ctx.enter_context(tc.tile_pool(name="osb", bufs=2))
    yfpool = ctx.enter_context(tc.tile_pool(name="yf", bufs=2))

    xT_t = xT.rearrange("(a p) t -> p a t", p=128)
    pend_trans = []  # deferred v transposes: (vst tile, chunk idx)

    def qkv_chunk(tci):
        """qkv projections for one 512-token chunk; v transposes deferred."""
        t0 = tci * TCHUNK
        xt = xpool.tile([128, KC, TCHUNK], BF16, tag="xt")
        for spl in range(4):
            eng = nc.sync if spl % 2 == 0 else nc.scalar
            eng.dma_start(
                xt[:, 2 * spl : 2 * spl + 2, :],
                xT_t[:, 2 * spl : 2 * spl + 2, t0 : t0 + TCHUNK],
            )
        for w_sb, bi, dst in ((wq_sb, 0, qT), (wk_sb, 1, kT), (wv_sb, 2, None)):
            ps = qkvps.tile([128, TCHUNK], F32, tag="qkvps")
            for a in range(KC):
                nc.tensor.matmul(
                    ps[:], lhsT=w_sb[:, a, :], rhs=xt[:, a, :],
                    start=(a == 0), stop=(a == KC - 1),
                )
            if dst is not None:
                nc.vector.tensor_scalar_add(
                    dst[:, t0 : t0 + TCHUNK], ps[:], b_sb[:, bi : bi + 1]
                )
            else:
                vst = vstp.tile([128, TCHUNK], BF16, tag="vst")
                nc.vector.tensor_scalar_add(vst[:], ps[:], b_sb[:, bi : bi + 1])
                pend_trans.append((vst, tci))

    def flush_trans():
        """PE-transpose pending v chunks into vsb (token-major)."""
        while pend_trans:
            vst, tci = pend_trans.pop(0)
            tpf = qkvps.tile([128, TCHUNK], F32, tag="qkvps")
            tp = tpf.bitcast(BF16)  # [128, 1024] bf16 view; use first 512
            for g4 in range(4):
                g = 4 * tci + g4
                nc.tensor.transpose(
                    tp[:, g4 * 128 : (g4 + 1) * 128],
                    vst[:, g4 * 128 : (g4 + 1) * 128], id_sb[:]
                )
                nc.vector.tensor_copy(
                    vsb[:, g, 0:HPC, 0:HD], tp[:, g4 * 128 : (g4 + 1) * 128]
                )

    def attention_batch(b):
        for qb in range(NQB):
            q0g = b * T + qb * QB
            njt = 4 * (qb + 1)
            yps = [
                ypool.tile([128, QB], F32, tag=f"yps{h}", name=f"yp{b}_{qb}_{h}")
                for h in range(HPC)
            ]
            for j in range(njt):
                j0g = b * T + j * 128
                i0 = max(0, j * 128 - qb * QB)
                diag = j * 128 + 127 > qb * QB
                sp = spool.tile([128, HPC, QB], F32, tag="sp")
                pt = ptpool.tile([128, HPC, QB], BF16, tag="pt")
                for h in range(HPC):
                    hs = slice(h * HD, (h + 1) * HD)
                    nc.tensor.matmul(
                        sp[:, h, i0:QB], lhsT=kT[hs, j0g : j0g + 128],
                        rhs=qT[hs, q0g + i0 : q0g + QB], start=True, stop=True,
                    )
                # one exp covers both heads (identical i0 geometry)
                nc.scalar.activation(
                    pt[:, :, i0:QB], sp[:, :, i0:QB], AF.Exp, scale=float(SCALE)
                )
                for h in range(HPC):
                    if diag:
                        nc.gpsimd.affine_select(
                            pt[:, h, i0 : i0 + 128], pt[:, h, i0 : i0 + 128],
                            pattern=[[1, 128]], base=0, channel_multiplier=-1,
                            compare_op=mybir.AluOpType.is_ge, fill=0.0,
                        )
                    nc.tensor.matmul(
                        yps[h][0 : HD + 1, i0:QB],
                        lhsT=vsb[:, b * JTN + j, h, 0 : HD + 1],
                        rhs=pt[:, h, i0:QB],
                        start=(j == 0), stop=(j == njt - 1),
                        skip_group_check=True,
                    )
            # evict raw y, reciprocal the denominators, stage for the collective
            yst = stpool.tile([FPC, QB], BF16, tag="yst")
            ln = stpool.tile([1, HPC, QB], F32, tag="ln", bufs=1)
            for h in range(HPC):
                nc.vector.tensor_copy(yst[h * HD : (h + 1) * HD, :], yps[h][0:HD, :])
                nc.vector.tensor_copy(ln[0:1, h, :], yps[h][HD : HD + 1, :])
            rnl = stpool.tile([1, HPC, QB], F32, tag="rnl", bufs=1)
            scr = stpool.tile([1, HPC, QB], F32, tag="scr", bufs=1)
            nc.vector.reciprocal_approx_accurate(rnl[:], ln[:], scr[:])
            rnb = rnl.bitcast(BF16)  # [1, HPC, 2*QB]
            for s in range(2):
                r = 2 * qb + s
                nc.sync.dma_start(
                    cc_in[b][r, 0:FPC, :], yst[:, s * STRIPE : (s + 1) * STRIPE]
                )
                # fp32 recips ride as 2 bf16 rows per head
                nc.sync.dma_start(
                    cc_in[b][r, FPC:CCR, :], rnb[0:1, :, s * QB : (s + 1) * QB]
                )
        nc.gpsimd.collective_compute(
            "AllToAll", mybir.AluOpType.bypass,
            ins=[cc_in[b][:]], outs=[cc_out[b][:]],
            replica_groups=[list(range(N_CORES))],
        )

    def project_batch(b):
        yfull = yfpool.tile([128, KC, STRIPE], BF16, tag="yf")
        nc.sync.dma_start(yfull[:], cc_out[b][:, 0:FPC, :].rearrange("r p t -> p r t"))
        # rebuild the per-(feature, t) scale tile by broadcast-DMAing the
        # fp32 reciprocals straight out of cc_out (partition-stride-0 APs)
        sc = yfpool.tile([128, KC, STRIPE], F32, tag="sc", bufs=1)
        for r in range(N_CORES):
            for h in range(HPC):
                row = cc_out[b][r, FPC + 2 * h : FPC + 2 * h + 2, :]
                rowf = row.rearrange("h t -> (h t)").bitcast(F32)  # [STRIPE] f32
                src = bass.AP(
                    tensor=rowf.tensor, offset=rowf.offset,
                    ap=[[0, HD]] + [list(p) for p in rowf.ap],
                )
                nc.sync.dma_start(sc[h * HD : (h + 1) * HD, r, :], src)
        nc.vector.tensor_mul(yfull[:], yfull[:], sc[:])
        for tt in range(STRIPE // 128):
            ps0 = ypool.tile([128, 512], F32, tag="yps0", name=f"pj{b}_{tt}_0")
            ps1 = ypool.tile([128, 512], F32, tag="yps1", name=f"pj{b}_{tt}_1")
            for a in range(KC):
                lhsT = yfull[:, a, tt * 128 : (tt + 1) * 128]
                nc.tensor.matmul(ps0[:], lhsT=lhsT, rhs=wp_sb[:, a, 0:512],
                                 start=(a == 0), stop=(a == KC - 1))
                nc.tensor.matmul(ps1[:], lhsT=lhsT, rhs=wp_sb[:, a, 512:C],
                                 start=(a == 0), stop=(a == KC - 1))
            osb = ospool.tile([128, C], F32, tag="osb")
            nc.vector.tensor_add(osb[:, 0:512], ps0[:], bp_sb[:, 0:512])
            nc.vector.tensor_add(osb[:, 512:C], ps1[:], bp_sb[:, 512:C])
            r0 = b * STRIPE + tt * 128
            nc.sync.dma_start(out[r0 : r0 + 128, :], osb[:])

    # ---------- software-pipelined emission ----------
    # qkv(b) -> [attention(b-1) interleaves via deps] ; proj(b) sits behind
    # attention(b+1) so no engine queue waits on an in-flight collective.
    for b in range(B):
        for tci in range(CPB * b, CPB * (b + 1)):
            qkv_chunk(tci)
        flush_trans()
        if b == 0:
            # deferred weight loads: keep the early DMA queues clear for x
            nc.sync.dma_start(wp_sb[:], wp.rearrange("(a p) e -> p a e", p=128))
            bp_bcast = bass.AP(
                tensor=bp.tensor, offset=bp.offset, ap=[[0, 128], [1, C]]
            )
            nc.sync.dma_start(bp_sb[:], bp_bcast)
        if b >= 1:
            attention_batch(b - 1)
        if b >= 2:
            project_batch(b - 2)
    attention_batch(B - 1)
    project_batch(B - 2)
    project_batch(B - 1)

    if dbg is not None:
        nc.sync.dma_start(dbg["d_qT"][:], qT[:])
        nc.sync.dma_start(dbg["d_kT"][:], kT[:])
        nc.sync.dma_start(dbg["d_vsb"][:], vsb[:, :, :, 0 : HD + 1])
        for b in range(B):
            nc.sync.dma_start(dbg["d_cc"][b], cc_in[b][:])
            nc.sync.dma_start(dbg["d_ccout"][b], cc_out[b][:])


_COMPILED_NC = None


def _get_nc():
    global _COMPILED_NC
    if _COMPILED_NC is None:
        nc = bacc.Bacc("TRN2", target_bir_lowering=False, debug=False,
                       num_devices=N_CORES)
        build_program(nc)
        nc.compile()
        _COMPILED_NC = nc
    return _COMPILED_NC


def kernel(x, W_attn, b_attn, W_proj, b_proj):
    global LAST_RESULTS
    nc = _get_nc()

    bf = ml_dtypes.bfloat16
    xT_np = np.ascontiguousarray(
        np.asarray(x, np.float32).reshape(BT, C).T
    ).astype(bf)
    W_attn = np.asarray(W_attn, np.float32)
    b_attn = np.asarray(b_attn, np.float32)
    wp_np = np.asarray(W_proj, np.float32).astype(bf)
    bp_np = np.asarray(b_proj, np.float32)
    id_np = np.eye(128, dtype=np.float32).astype(bf)

    in_maps = []
    for c in range(N_CORES):
        s = slice(c * FPC, (c + 1) * FPC)
        in_maps.append({
            "xT": xT_np,
            "wq": np.ascontiguousarray(W_attn[:, s]).astype(bf),
            "wk": np.ascontiguousarray(W_attn[:, C:2 * C][:, s]).astype(bf),
            "wv": np.ascontiguousarray(W_attn[:, 2 * C:][:, s]).astype(bf),
            "bqkv": np.ascontiguousarray(
                np.stack([b_attn[s], b_attn[C:2 * C][s], b_attn[2 * C:][s]])
            ).astype(np.float32),
            "wp": wp_np,
            "bp": bp_np,
            "ident": id_np,
        })

    res = run_bass_kernel_spmd(nc, in_maps, core_ids=list(range(N_CORES)))
    LAST_RESULTS = res
    # core r, out row b*256+i  ->  full row b*2048 + r*256 + i
    outs = np.stack([res.results[c]["out"] for c in range(N_CORES)])
    full = outs.reshape(N_CORES, B, STRIPE, C).transpose(1, 0, 2, 3)
    return np.ascontiguousarray(full.reshape(B, T, C))


# revision 34
# speedup vs baseline: 1.2373x; 1.0282x over previous
"""Causal self-attention (GPT-style block) on 8 Trainium2 NeuronCores.

Problem: x[4,2048,1024] -> qkv = x@W_attn+b ; 16-head causal attention
(head_dim 64) ; out = y@W_proj+b_proj.

Sharding: tensor-parallel over heads. Core c owns heads {2c, 2c+1}:
  - qkv projections from a host-pretransposed x^T (bf16); q^T/k^T stay
    feature-major for the S matmul; v is rotated to token-major layout
    on the PE via transpose-matmuls (no DMA transposes),
  - causal attention in S^T layout: scores [128 j-keys, 512 queries].
    Both heads of a j-tile share one 2-bank PSUM tile so a single
    ScalarE exp instruction covers them; diagonal masking via GpSimd
    affine_select; PV appends a ones-column to V so the softmax
    denominator falls out of the same PSUM tile. Normalization is
    deferred past the collective: raw y and the denominators travel
    together (130 bf16 rows per stripe).
  - per-batch striped AllToAll; emission is software-pipelined so batch
    b's collective+projection instructions sit behind batch b+1's
    attention in every engine queue (in-order queues never stall on an
    in-flight collective).

Numerics: bf16 operands, fp32 PSUM accumulation; softmax skips
max-subtraction (scores are O(1); exp stays finite).
"""

import numpy as np
import ml_dtypes
from contextlib import ExitStack

import concourse.bass as bass
import concourse.tile as tile
from concourse import bacc, mybir
from concourse.bass_utils import run_bass_kernel_spmd

F32 = mybir.dt.float32
BF16 = mybir.dt.bfloat16
AF = mybir.ActivationFunctionType

N_CORES = 8
B, T, C, H = 4, 2048, 1024, 16
HD = C // H            # 64 head dim
HPC = H // N_CORES     # 2 heads per core
FPC = HPC * HD         # 128 features per core
BT = B * T             # 8192 rows
TCHUNK = 512           # t chunk in qkv phase
NT_CHUNKS = BT // TCHUNK
CPB = T // TCHUNK      # 4 chunks per batch
QB = 512               # query block
NQB = T // QB          # 4 per batch
JTN = T // 128         # 16 j-tiles per batch
ROWS = BT // N_CORES   # 1024 rows per core after AllToAll
KC = C // 128          # 8 contraction tiles over C
STRIPE = 256           # rows per (core, batch) stripe
CCR = FPC + 2 * HPC    # A2A payload rows: y (128) + fp32 recip denoms (2x2 bf16)
SCALE = 1.0 / np.sqrt(HD)

LAST_RESULTS = None    # test.py reads exec_time_ns off this


def build_program(nc, debug=False):
    xT = nc.dram_tensor("xT", [C, BT], BF16, kind="ExternalInput").ap()
    wq = nc.dram_tensor("wq", [C, FPC], BF16, kind="ExternalInput").ap()
    wk = nc.dram_tensor("wk", [C, FPC], BF16, kind="ExternalInput").ap()
    wv = nc.dram_tensor("wv", [C, FPC], BF16, kind="ExternalInput").ap()
    bqkv = nc.dram_tensor("bqkv", [3, FPC], F32, kind="ExternalInput").ap()
    wp = nc.dram_tensor("wp", [C, C], BF16, kind="ExternalInput").ap()
    bp = nc.dram_tensor("bp", [C], F32, kind="ExternalInput").ap()
    ident = nc.dram_tensor("ident", [128, 128], BF16, kind="ExternalInput").ap()
    out = nc.dram_tensor("out", [ROWS, C], F32, kind="ExternalOutput").ap()
    cc_in = [
        nc.dram_tensor(f"cc_in{b}", [N_CORES, CCR, STRIPE], BF16, kind="Internal").ap()
        for b in range(B)
    ]
    cc_out = [
        nc.dram_tensor(f"cc_out{b}", [N_CORES, CCR, STRIPE], BF16, kind="Internal").ap()
        for b in range(B)
    ]

    dbg = None
    if debug:
        dbg = {
            "d_qT": nc.dram_tensor("d_qT", [128, BT], BF16, kind="ExternalOutput").ap(),
            "d_kT": nc.dram_tensor("d_kT", [128, BT], BF16, kind="ExternalOutput").ap(),
            "d_vsb": nc.dram_tensor(
                "d_vsb", [128, B * JTN, HPC, HD + 1], BF16, kind="ExternalOutput"
            ).ap(),
            "d_cc": nc.dram_tensor(
                "d_cc", [B, N_CORES, CCR, STRIPE], BF16, kind="ExternalOutput"
            ).ap(),
            "d_ccout": nc.dram_tensor(
                "d_ccout", [B, N_CORES, CCR, STRIPE], BF16, kind="ExternalOutput"
            ).ap(),
        }
    with tile.TileContext(nc) as tc:
        with ExitStack() as ctx:
            emit(ctx, tc, xT, wq, wk, wv, bqkv, wp, bp, ident, out, cc_in, cc_out, dbg)
    return nc


def emit(ctx, tc, xT, wq, wk, wv, bqkv, wp, bp, ident, out, cc_in, cc_out, dbg=None):
    nc = tc.nc
    res = ctx.enter_context(tc.tile_pool(name="resident", bufs=1))

    # ---------- resident SBUF ----------
    qT = res.tile([128, BT], BF16)
    kT = res.tile([128, BT], BF16)
    vsb = res.tile([128, B * JTN, HPC, HD + 2], BF16)  # v natural + ones col
    wq_sb = res.tile([128, KC, FPC], BF16)
    wk_sb = res.tile([128, KC, FPC], BF16)
    wv_sb = res.tile([128, KC, FPC], BF16)
    b_sb = res.tile([128, 3], F32)
    id_sb = res.tile([128, 128], BF16)
    wp_sb = res.tile([128, KC, C], BF16)
    bp_sb = res.tile([128, C], F32)

    nc.scalar.dma_start(wq_sb[:], wq.rearrange("(a p) m -> p a m", p=128))
    nc.scalar.dma_start(wk_sb[:], wk.rearrange("(a p) m -> p a m", p=128))
    nc.scalar.dma_start(wv_sb[:], wv.rearrange("(a p) m -> p a m", p=128))
    nc.scalar.dma_start(b_sb[:], bqkv.rearrange("b p -> p b"))
    nc.scalar.dma_start(id_sb[:], ident)
    nc.vector.memset(vsb[:, :, :, HD : HD + 1], 1.0)

    # ---------- pools ----------
    # PSUM budget (8 banks): qkvps ring 2 (qkv chains + v transposes via
    # bitcast) + sp 4 (2-bank head-pair tiles, double-buffered) + ypool 2
    # (PV accumulators, reused as proj accumulators).
    xpool = ctx.enter_context(tc.tile_pool(name="xt", bufs=5))
    qkvps = ctx.enter_context(tc.tile_pool(name="qkvps", bufs=2, space="PSUM"))
    vstp = ctx.enter_context(tc.tile_pool(name="vst", bufs=5))
    spool = ctx.enter_context(tc.tile_pool(name="sps", bufs=2, space="PSUM"))
    ypool = ctx.enter_context(tc.tile_pool(name="yps", bufs=1, space="PSUM"))
    ptpool = ctx.enter_context(tc.tile_pool(name="pt", bufs=4))
    stpool = ctx.enter_context(tc.tile_pool(name="stg", bufs=3))
    ospool = ctx.enter_context(tc.tile_pool(name="osb", bufs=2))
    yfpool = ctx.enter_context(tc.tile_pool(name="yf", bufs=2))

    xT_t = xT.rearrange("(a p) t -> p a t", p=128)
    pend_trans = []  # deferred v transposes: (vst tile, chunk idx)

    def qkv_chunk(tci):
        """qkv projections for one 512-token chunk; v transposes deferred."""
        t0 = tci * TCHUNK
        xt = xpool.tile([128, KC, TCHUNK], BF16, tag="xt")
        for spl in range(4):
            eng = nc.sync if spl % 2 == 0 else nc.scalar
            eng.dma_start(
                xt[:, 2 * spl : 2 * spl + 2, :],
                xT_t[:, 2 * spl : 2 * spl + 2, t0 : t0 + TCHUNK],
            )
        for w_sb, bi, dst in ((wq_sb, 0, qT), (wk_sb, 1, kT), (wv_sb, 2, None)):
            ps = qkvps.tile([128, TCHUNK], F32, tag="qkvps")
            for a in range(KC):
                nc.tensor.matmul(
                    ps[:], lhsT=w_sb[:, a, :], rhs=xt[:, a, :],
                    start=(a == 0), stop=(a == KC - 1),
                )
            if dst is not None:
                nc.vector.tensor_scalar_add(
                    dst[:, t0 : t0 + TCHUNK], ps[:], b_sb[:, bi : bi + 1]
                )
            else:
                vst = vstp.tile([128, TCHUNK], BF16, tag="vst")
                nc.vector.tensor_scalar_add(vst[:], ps[:], b_sb[:, bi : bi + 1])
                pend_trans.append((vst, tci))

    def flush_trans():
        """PE-transpose pending v chunks into vsb (token-major)."""
        while pend_trans:
            vst, tci = pend_trans.pop(0)
            tpf = qkvps.tile([128, TCHUNK], F32, tag="qkvps")
            tp = tpf.bitcast(BF16)  # [128, 1024] bf16 view; use first 512
            for g4 in range(4):
                g = 4 * tci + g4
                nc.tensor.transpose(
                    tp[:, g4 * 128 : (g4 + 1) * 128],
                    vst[:, g4 * 128 : (g4 + 1) * 128], id_sb[:]
                )
                nc.vector.tensor_copy(
                    vsb[:, g, 0:HPC, 0:HD], tp[:, g4 * 128 : (g4 + 1) * 128]
                )

    def attention_batch(b):
        for qb in range(NQB):
            q0g = b * T + qb * QB
            njt = 4 * (qb + 1)
            yps = [
                ypool.tile([128, QB], F32, tag=f"yps{h}", name=f"yp{b}_{qb}_{h}")
                for h in range(HPC)
            ]
            for j in range(njt):
                j0g = b * T + j * 128
                i0 = max(0, j * 128 - qb * QB)
                diag = j * 128 + 127 > qb * QB
                sp = spool.tile([128, HPC, QB], F32, tag="sp")
                pt = ptpool.tile([128, HPC, QB], BF16, tag="pt")
                for h in range(HPC):
                    hs = slice(h * HD, (h + 1) * HD)
                    nc.tensor.matmul(
                        sp[:, h, i0:QB], lhsT=kT[hs, j0g : j0g + 128],
                        rhs=qT[hs, q0g + i0 : q0g + QB], start=True, stop=True,
                    )
                # one exp covers both heads (identical i0 geometry)
                nc.scalar.activation(
                    pt[:, :, i0:QB], sp[:, :, i0:QB], AF.Exp, scale=float(SCALE)
                )
                for h in range(HPC):
                    if diag:
                        nc.gpsimd.affine_select(
                            pt[:, h, i0 : i0 + 128], pt[:, h, i0 : i0 + 128],
                            pattern=[[1, 128]], base=0, channel_multiplier=-1,
                            compare_op=mybir.AluOpType.is_ge, fill=0.0,
                        )
                    nc.tensor.matmul(
                        yps[h][0 : HD + 1, i0:QB],
                        lhsT=vsb[:, b * JTN + j, h, 0 : HD + 1],
                        rhs=pt[:, h, i0:QB],
                        start=(j == 0), stop=(j == njt - 1),
                        skip_group_check=True,
                    )
            # evict raw y, reciprocal the denominators, stage for the collective
            yst = stpool.tile([FPC, QB], BF16, tag="yst")
            ln = stpool.tile([1, HPC, QB], F32, tag="ln", bufs=1)
            for h in range(HPC):
                nc.vector.tensor_copy(yst[h * HD : (h + 1) * HD, :], yps[h][0:HD, :])
                nc.vector.tensor_copy(ln[0:1, h, :], yps[h][HD : HD + 1, :])
            rnl = stpool.tile([1, HPC, QB], F32, tag="rnl", bufs=1)
            scr = stpool.tile([1, HPC, QB], F32, tag="scr", bufs=1)
            nc.vector.reciprocal_approx_accurate(rnl[:], ln[:], scr[:])
            rnb = rnl.bitcast(BF16)  # [1, HPC, 2*QB]
            for s in range(2):
                r = 2 * qb + s
                nc.sync.dma_start(
                    cc_in[b][r, 0:FPC, :], yst[:, s * STRIPE : (s + 1) * STRIPE]
                )
                # fp32 recips ride as 2 bf16 rows per head
                nc.sync.dma_start(
                    cc_in[b][r, FPC:CCR, :], rnb[0:1, :, s * QB : (s + 1) * QB]
                )
        nc.gpsimd.collective_compute(
            "AllToAll", mybir.AluOpType.bypass,
            ins=[cc_in[b][:]], outs=[cc_out[b][:]],
            replica_groups=[list(range(N_CORES))],
        )

    def project_batch(b):
        yfull = yfpool.tile([128, KC, STRIPE], BF16, tag="yf")
        nc.sync.dma_start(yfull[:], cc_out[b][:, 0:FPC, :].rearrange("r p t -> p r t"))
        # rebuild the per-(feature, t) scale tile with ONE broadcast DMA:
        # dst enumerates (p=h*64+d, r, t); src walks the f32-bitcast recip
        # rows of cc_out with a stride-0 d-dim.
        sc = yfpool.tile([128, KC, STRIPE], F32, tag="sc", bufs=1)
        flat = cc_out[b][:].rearrange("r c t -> (r c t)").bitcast(F32)
        rs_f32 = CCR * STRIPE // 2
        for h in range(HPC):
            srcap = bass.AP(
                tensor=flat.tensor,
                offset=flat.offset + (FPC + 2 * h) * STRIPE // 2,
                ap=[[0, HD], [rs_f32, N_CORES], [1, STRIPE]],
            )
            nc.sync.dma_start(sc[h * HD : (h + 1) * HD, :, :], srcap)
        nc.vector.tensor_mul(yfull[:], yfull[:], sc[:])
        for tt in range(STRIPE // 128):
            ps0 = ypool.tile([128, 512], F32, tag="yps0", name=f"pj{b}_{tt}_0")
            ps1 = ypool.tile([128, 512], F32, tag="yps1", name=f"pj{b}_{tt}_1")
            for a in range(KC):
                lhsT = yfull[:, a, tt * 128 : (tt + 1) * 128]
                nc.tensor.matmul(ps0[:], lhsT=lhsT, rhs=wp_sb[:, a, 0:512],
                                 start=(a == 0), stop=(a == KC - 1))
                nc.tensor.matmul(ps1[:], lhsT=lhsT, rhs=wp_sb[:, a, 512:C],
                                 start=(a == 0), stop=(a == KC - 1))
            osb = ospool.tile([128, C], F32, tag="osb")
            nc.vector.tensor_add(osb[:, 0:512], ps0[:], bp_sb[:, 0:512])
            nc.vector.tensor_add(osb[:, 512:C], ps1[:], bp_sb[:, 512:C])
            r0 = b * STRIPE + tt * 128
            nc.sync.dma_start(out[r0 : r0 + 128, :], osb[:])

    # ---------- software-pipelined emission ----------
    # qkv(b) -> [attention(b-1) interleaves via deps] ; proj(b) sits behind
    # attention(b+1) so no engine queue waits on an in-flight collective.
    for b in range(B):
        for tci in range(CPB * b, CPB * (b + 1)):
            qkv_chunk(tci)
        flush_trans()
        if b == 0:
            # deferred weight loads: keep the early DMA queues clear for x
            nc.sync.dma_start(wp_sb[:], wp.rearrange("(a p) e -> p a e", p=128))
            bp_bcast = bass.AP(
                tensor=bp.tensor, offset=bp.offset, ap=[[0, 128], [1, C]]
            )
            nc.sync.dma_start(bp_sb[:], bp_bcast)
        if b >= 1:
            attention_batch(b - 1)
        if b >= 2:
            project_batch(b - 2)
    attention_batch(B - 1)
    project_batch(B - 2)
    project_batch(B - 1)

    if dbg is not None:
        nc.sync.dma_start(dbg["d_qT"][:], qT[:])
        nc.sync.dma_start(dbg["d_kT"][:], kT[:])
        nc.sync.dma_start(dbg["d_vsb"][:], vsb[:, :, :, 0 : HD + 1])
        for b in range(B):
            nc.sync.dma_start(dbg["d_cc"][b], cc_in[b][:])
            nc.sync.dma_start(dbg["d_ccout"][b], cc_out[b][:])


_COMPILED_NC = None


def _get_nc():
    global _COMPILED_NC
    if _COMPILED_NC is None:
        nc = bacc.Bacc("TRN2", target_bir_lowering=False, debug=False,
                       num_devices=N_CORES)
        build_program(nc)
        nc.compile()
        _COMPILED_NC = nc
    return _COMPILED_NC


def kernel(x, W_attn, b_attn, W_proj, b_proj):
    global LAST_RESULTS
    nc = _get_nc()

    bf = ml_dtypes.bfloat16
    xT_np = np.ascontiguousarray(
        np.asarray(x, np.float32).reshape(BT, C).T
    ).astype(bf)
    W_attn = np.asarray(W_attn, np.float32)
    b_attn = np.asarray(b_attn, np.float32)
    wp_np = np.asarray(W_proj, np.float32).astype(bf)
    bp_np = np.asarray(b_proj, np.float32)
    id_np = np.eye(128, dtype=np.float32).astype(bf)

    in_maps = []
    for c in range(N_CORES):
        s = slice(c * FPC, (c + 1) * FPC)
        in_maps.append({
            "xT": xT_np,
            "wq": np.ascontiguousarray(W_attn[:, s]).astype(bf),
            "wk": np.ascontiguousarray(W_attn[:, C:2 * C][:, s]).astype(bf),
            "wv": np.ascontiguousarray(W_attn[:, 2 * C:][:, s]).astype(bf),
            "bqkv": np.ascontiguousarray(
                np.stack([b_attn[s], b_attn[C:2 * C][s], b_attn[2 * C:][s]])
            ).astype(np.float32),
            "wp": wp_np,
            "bp": bp_np,
            "ident": id_np,
        })

    res = run_bass_kernel_spmd(nc, in_maps, core_ids=list(range(N_CORES)))
    LAST_RESULTS = res
    # core r, out row b*256+i  ->  full row b*2048 + r*256 + i
    outs = np.stack([res.results[c]["out"] for c in range(N_CORES)])
    full = outs.reshape(N_CORES, B, STRIPE, C).transpose(1, 0, 2, 3)
    return np.ascontiguousarray(full.reshape(B, T, C))
